# revision 20
# baseline (speedup 1.0000x reference)
"""DeeperGCN kernel for 8 TRN2 NeuronCores (bass SPMD).

Graph/data-parallel over 8 cores: nodes sharded at graph boundaries (batch is
sorted); within each core nodes are re-ordered by in-degree so the softmax
aggregation becomes "plane" accumulation (plane j = the j-th incoming edge of
every node; degree-sorted nodes make every plane a contiguous node prefix, so
aggregation is a few wide vector adds per layer instead of per-edge-tile
scatter matmuls). Feature-major [D, nodes] resident state on-chip; one
AllGather of h2 per layer serves cross-shard edges; exact global BatchNorm via
small AllReduces. The plane structure is uniformized across cores (SPMD: one
program); per-core variability lives entirely in data (gather indices,
sentinel rows, pad masks, dummy-count corrections).

Falls back to a scipy host implementation on any device failure.
"""
import os
import sys
import time
import traceback

import numpy as np

sys.path.insert(0, '/opt/trn_rl_repo')

N_LAYERS = 20
EPS = np.float32(1e-7)
BN_EPS = np.float32(1e-5)
N, E, G, D, H = 100000, 400000, 4096, 128, 256
NC = 8
GPC = G // NC
SC = 2048              # edge superchunk (slots)
CH = 512               # matmul chunk
LAST_HW_EXEC_NS = None
_CACHE = {}


# ---------------------------------------------------------------------------
# host fallback
# ---------------------------------------------------------------------------

def _host_reference(x, edge_index, edge_attr, batch, atom_emb, bond_emb, W1, b1,
                    g1, be1, W2, b2, g2, be2, W3, b3, t, norm_g, norm_b, predW,
                    predb, n_layers=N_LAYERS):
    from scipy.sparse import csr_matrix
    src, dst = edge_index[0], edge_index[1]
    ne = src.shape[0]
    A = csr_matrix((np.ones(ne, np.float32), (dst, np.arange(ne))), shape=(N, ne))

    def bn(xx, g, b):
        mu = xx.mean(0, dtype=np.float32)
        var = xx.var(0, dtype=np.float32)
        return (xx - mu) * (1.0 / np.sqrt(var + BN_EPS)) * g + b

    h = atom_emb[0][x[:, 0]].copy()
    for i in range(1, 9):
        h += atom_emb[i][x[:, i]]
    code = (edge_attr[:, 0] * 36 + edge_attr[:, 1] * 6 + edge_attr[:, 2]).astype(np.int64)
    for l in range(n_layers):
        h2 = np.maximum(bn(h, norm_g[l - 1], norm_b[l - 1]), 0) if l else h
        bl = bond_emb[l]
        C216 = (bl[0][:, None, None, :] + bl[1][None, :, None, :]
                + bl[2][None, None, :, :]).reshape(216, D)
        msg = np.maximum(h2[src] + C216[code], 0)
        ex = np.exp(msg * t[l])
        m = (A @ (msg * ex)) / np.maximum(A @ ex, EPS)
        hh = h2 + m
        a = np.maximum(bn(hh @ W1[l] + b1[l], g1[l], be1[l]), 0)
        a = np.maximum(bn(a @ W2[l] + b2[l], g2[l], be2[l]), 0)
        out = a @ W3[l] + b3[l]
        h = out + h if l else out
    h = bn(h, norm_g[n_layers - 1], norm_b[n_layers - 1])
    cnt = np.bincount(batch, minlength=G).astype(np.float32)
    sums = np.zeros((G, D), np.float32)
    np.add.at(sums, batch, h)
    hg = sums / np.maximum(cnt, 1.0)[:, None]
    return (hg @ predW + predb).astype(np.float32)


# ---------------------------------------------------------------------------
# host preprocessing
# ---------------------------------------------------------------------------

def _plane_segments(offs, njs, lo, hi):
    out = []
    for j in range(len(njs)):
        a, b = max(offs[j], lo), min(offs[j] + njs[j], hi)
        if a < b:
            out.append((a - lo, a - offs[j], b - a))
    return out


def _preprocess(inputs, n_layers):
    x = inputs['x']; edge_index = inputs['edge_index']
    edge_attr = inputs['edge_attr']; batch = inputs['batch']
    assert x.min() >= 0 and x.max() <= 1
    assert np.all(inputs['t'][:n_layers] > 0)
    src_g = edge_index[0].astype(np.int64)
    dst_g = edge_index[1].astype(np.int64)

    cuts = np.searchsorted(batch, np.arange(0, G + 1, GPC)).astype(np.int64)
    nlocs = np.diff(cuts)
    npad = 13312
    sent = npad - 128
    assert nlocs.max() <= sent - 1, nlocs.max()
    owner = np.searchsorted(cuts[1:], np.arange(N), side='right')

    indeg = np.bincount(dst_g, minlength=N)
    rank = np.empty(N, np.int64)
    core_deg_sorted = []
    for c in range(NC):
        n0, n1 = cuts[c], cuts[c + 1]
        deg = indeg[n0:n1]
        order = np.argsort(-deg, kind='stable')
        rank[n0 + order] = np.arange(n1 - n0)
        core_deg_sorted.append(deg[order])
    gpid = (owner * npad + rank).astype(np.int64)

    maxdeg = max(int(d[0]) if len(d) else 0 for d in core_deg_sorted)
    nj_core = np.zeros((NC, maxdeg), np.int64)
    for c in range(NC):
        d = core_deg_sorted[c]
        for j in range(maxdeg):
            nj_core[c, j] = int((d > j).sum())
    nj = nj_core.max(0)                       # uniform plane sizes
    offs = np.concatenate([[0], np.cumsum(nj)]).astype(np.int64)
    slots_used = int(offs[-1])
    slots_pad = -(-slots_used // CH) * CH
    nsc = -(-slots_pad // SC)
    segs = [_plane_segments(offs, nj, s * SC, min((s + 1) * SC, slots_pad))
            for s in range(nsc)]

    code = (edge_attr[:, 0] * 36 + edge_attr[:, 1] * 6 + edge_attr[:, 2]).astype(np.int64)
    ucodes = np.unique(code)
    assert len(ucodes) <= 8
    code_id = np.searchsorted(ucodes, code)

    # pooling structure (uniform)
    gsz_all = np.diff(np.searchsorted(batch, np.arange(G + 1))).astype(np.int64)
    gsz_c = gsz_all.reshape(NC, GPC)
    gorder_c = np.argsort(-gsz_c, axis=1, kind='stable')
    gsorted_c = np.take_along_axis(gsz_c, gorder_c, 1)
    gmax = int(gsorted_c.max())
    pnj = np.array([ (gsorted_c > j).sum(1).max() for j in range(gmax) ], np.int64)
    poffs = np.concatenate([[0], np.cumsum(pnj)]).astype(np.int64)
    pslots_used = int(poffs[-1])
    pslots_pad = -(-pslots_used // 128) * 128
    pnsc = -(-pslots_pad // SC)
    psegs = [_plane_segments(poffs, pnj, s * SC, min((s + 1) * SC, pslots_pad))
             for s in range(pnsc)]
    ptg = pslots_pad // 128

    tg = slots_pad // 128
    nchunk = npad // CH

    cores = []
    for c in range(NC):
        n0, n1 = cuts[c], cuts[c + 1]
        nloc = int(n1 - n0)
        emask = owner[dst_g] == c
        e_src = src_g[emask]; e_dst = dst_g[emask]; e_cid = code_id[emask]
        r = rank[e_dst]
        es = np.argsort(r, kind='stable')
        e_src, e_cid, r = e_src[es], e_cid[es], r[es]
        bc = np.bincount(r, minlength=nloc)
        occ = np.arange(len(r)) - np.concatenate([[0], np.cumsum(bc)])[r]
        slot = offs[occ] + r

        gsrc = np.full(slots_pad, c * npad + sent, np.int32)   # sentinel
        gsrc[slot] = gpid[e_src].astype(np.int32)
        cid = np.full(slots_pad, -1, np.int64)
        cid[slot] = e_cid
        gidx = gsrc.reshape(tg, 128).T.copy()

        oh = np.zeros((nsc, 128, 2 * CH), np.float32)
        for ck in range(slots_pad // CH):
            s, cc = ck // (SC // CH), ck % (SC // CH)
            bse = 32 * (cc % 4)
            cs = (cc // 4) * CH
            ids = cid[ck * CH:(ck + 1) * CH]
            jj = np.nonzero(ids >= 0)[0]
            oh[s, bse + ids[jj], cs + jj] = 1.0

        # dummy-count per node column (uniform planes minus real edges)
        dc = np.zeros(npad, np.float32)
        for j in range(maxdeg):
            a, b = int(nj_core[c, j]), int(nj[j])
            if a < b:
                dc[a:b] += 1.0
        dneg = np.broadcast_to(-dc, (128, npad)).copy()


        # pooling indices
        gstart = np.searchsorted(batch[n0:n1], np.arange(c * GPC, (c + 1) * GPC))
        pidx = np.full(pslots_pad, sent, np.int32)
        go = gorder_c[c]; gs = gsorted_c[c]
        for j in range(gmax):
            ng = int((gs > j).sum())
            sel = go[:ng]
            pidx[poffs[j]:poffs[j] + ng] = rank[n0 + gstart[sel] + j].astype(np.int32)
        pool_idx = pidx.reshape(ptg, 128).T.copy()

        xt = np.zeros((16, npad), np.float32)
        xt[:9, rank[n0:n1]] = x[n0:n1].T.astype(np.float32)

        cores.append(dict(gidx=gidx, oh=oh, xt=xt, dneg=dneg,
                          nlv=np.full((128, 1), float(npad - nloc), np.float32),
                          pool_idx=pool_idx, nloc=nloc,
                          gorder=go, gsz=gsz_c[c]))

    ae = inputs['atom_emb'].astype(np.float32)
    base = ae[:, 0, :].sum(0)[:, None]              # [D,1]
    delta = np.zeros((16, D), np.float32)
    delta[:9] = ae[:, 1, :] - ae[:, 0, :]
    be_ = inputs['bond_emb'].astype(np.float32)
    a0 = ucodes // 36; a1c = (ucodes // 6) % 6; a2c = ucodes % 6
    c8 = np.zeros((n_layers, 8, D), np.float32)
    for l in range(n_layers):
        c8[l, :len(ucodes)] = be_[l, 0, a0] + be_[l, 1, a1c] + be_[l, 2, a2c]

    def pack2(v):   # [L,256] -> [L,128,2]
        return v.reshape(v.shape[0], 2, 128).transpose(0, 2, 1).copy()

    v1 = np.concatenate([pack2(inputs['b1'][:n_layers]),
                         pack2(inputs['g1'][:n_layers]),
                         pack2(inputs['be1'][:n_layers])], axis=2)
    v2 = np.concatenate([pack2(inputs['b2'][:n_layers]),
                         pack2(inputs['g2'][:n_layers]),
                         pack2(inputs['be2'][:n_layers])], axis=2)
    v3 = np.stack([inputs['b3'][:n_layers], inputs['norm_g'][:n_layers],
                   inputs['norm_b'][:n_layers]], axis=2)   # [L,128,3]

    meta = dict(npad=npad, sent=sent, slots_pad=slots_pad, nsc=nsc, segs=segs,
                tg=tg, nchunk=nchunk, ptg=ptg, psegs=psegs,
                pslots_pad=pslots_pad, pnsc=pnsc,
                tvals=[float(v) for v in inputs['t'][:n_layers]],
                n_layers=n_layers)
    shared = dict(base=base, delta=delta, c8=c8.reshape(n_layers * 8, D),
                  v1=v1, v2=v2, v3=v3,
                  w1=inputs['W1'][:n_layers], w2=inputs['W2'][:n_layers],
                  w3=inputs['W3'][:n_layers])
    return meta, shared, cores


# ---------------------------------------------------------------------------
# device program
# ---------------------------------------------------------------------------

def _build_program(meta):
    import contextlib
    from concourse import bass, bacc, mybir, tile
    dt = mybir.dt
    AF = mybir.ActivationFunctionType
    OP = mybir.AluOpType
    ts = bass.ts
    AX = mybir.AxisListType.X

    npad = meta['npad']; sent = meta['sent']
    slots_pad = meta['slots_pad']; nsc = meta['nsc']; segs = meta['segs']
    tg = meta['tg']; nchunk = meta['nchunk']
    ptg = meta['ptg']; psegs = meta['psegs']; pslots_pad = meta['pslots_pad']
    tvals = meta['tvals']; n_layers = meta['n_layers']
    INV_N = 1.0 / float(N)

    nc = bacc.Bacc("TRN2", target_bir_lowering=False, debug=False,
                   num_devices=NC)

    Idef = [
        ("xt", [16, npad], dt.bfloat16), ("gidx", [128, tg], dt.int32),
        ("oh", [nsc, 128, 2 * CH], dt.bfloat16),
        ("pool_idx", [128, ptg], dt.int32),
        ("dneg", [128, npad], dt.float32), ("nlv", [128, 1], dt.float32),
        ("delta", [16, 128], dt.bfloat16), ("base", [128, 1], dt.float32),
        ("c8", [n_layers * 8, 128], dt.bfloat16),
        ("w1", [n_layers, 128, 256], dt.float32),
        ("w2", [n_layers, 256, 256], dt.float32),
        ("w3", [n_layers, 256, 128], dt.float32),
        ("v1", [n_layers, 128, 6], dt.float32),
        ("v2", [n_layers, 128, 6], dt.float32),
        ("v3", [n_layers, 128, 3], dt.float32),
    ]
    I = {nm: nc.dram_tensor(nm, sh, d, kind="ExternalInput") for nm, sh, d in Idef}
    out_pool = nc.dram_tensor("out_pool", [128, GPC], dt.float32,
                              kind="ExternalOutput")

    with tile.TileContext(nc) as tc:
        ctx = contextlib.ExitStack()
        ctx.enter_context(nc.allow_low_precision(
            reason="bf16 edge pipeline validated against host"))
        sb = ctx.enter_context(tc.tile_pool(name="sb", bufs=1))
        wk = ctx.enter_context(tc.tile_pool(name="wk", bufs=2))
        gp = ctx.enter_context(tc.tile_pool(name="gp", bufs=1))
        st = ctx.enter_context(tc.tile_pool(name="st", bufs=1))
        ps = ctx.enter_context(tc.tile_pool(name="ps", bufs=4, space="PSUM"))
        p1 = ctx.enter_context(tc.tile_pool(name="p1", bufs=2, space="PSUM"))
        dr = ctx.enter_context(tc.tile_pool(name="dr", bufs=1, space="DRAM"))

        # rotating 52KB state slots: tag sl0 / sl1
        hhT = sb.tile([128, npad], dt.float16, name="hhT")
        gix = sb.tile([128, tg], dt.int32, name="gix")
        pix = sb.tile([128, ptg], dt.int32, name="pix")
        eps_t = sb.tile([128, 1], dt.float32, name="eps_t")
        stat = sb.tile([128, 8], dt.float32, name="stat")
        sp = sb.tile([128, 2, 32], dt.float32, name="sp")
        sq = sb.tile([128, 2, 32], dt.float32, name="sq")
        ab = sb.tile([128, 8], dt.float32, name="ab")
        c8S = sb.tile([128, 128], dt.bfloat16, name="c8S")
        w1S = sb.tile([128, 256], dt.float32, name="w1S")
        w2S = sb.tile([128, 2, 256], dt.float32, name="w2S")
        w3S = sb.tile([128, 2, 128], dt.float32, name="w3S")
        v1S = sb.tile([128, 6], dt.float32, name="v1S")
        v2S = sb.tile([128, 6], dt.float32, name="v2S")
        v3S = sb.tile([128, 3], dt.float32, name="v3S")
        v3P = sb.tile([128, 3], dt.float32, name="v3P")
        sentS = sb.tile([128, 128], dt.float16, name="sentS")
        nlvS = sb.tile([128, 1], dt.float32, name="nlvS")
        poolT = sb.tile([128, GPC], dt.float32, name="poolT")
        hpad = sb.tile([128, 1], dt.float32, name="hpad")
        h2pad = sb.tile([128, 1], dt.float32, name="h2pad")
        y1p = sb.tile([128, 2], dt.float32, name="y1p")
        y2p = sb.tile([128, 2], dt.float32, name="y2p")
        o3p = sb.tile([128, 1], dt.float32, name="o3p")
        ptmp = sb.tile([128, 2], dt.float32, name="ptmp")
        padb = sb.tile([128, 2], dt.bfloat16, name="padb")

        ag_in = dr.tile([npad, 128], dt.float16, name="ag_in")
        y1d = dr.tile([2, nchunk, 128, CH], dt.float32, name="y1d")
        y2d = dr.tile([2, nchunk, 128, CH], dt.float32, name="y2d")
        hdram = dr.tile([128, npad], dt.float32, name="hdram")
        gdram = dr.tile([SC, 128], dt.float16, name="gdram", bufs=2)
        arin = dr.tile([128, 4], dt.float32, name="arin", bufs=2)

        nc.sync.dma_start(gix[:], I['gidx'].ap())
        nc.sync.dma_start(pix[:], I['pool_idx'].ap())
        nc.sync.dma_start(nlvS[:], I['nlv'].ap())
        nc.vector.memset(eps_t[:], float(BN_EPS))
        nc.vector.memset(sentS[:], -1000.0)

        RG = [list(range(NC))]

        def fslot(i):
            return st.tile([128, npad], dt.float32, tag=f"sl{i % 2}",
                           name=f"f{i % 2}")

        def bslot(i):
            return (st.tile([128, npad], dt.bfloat16, tag=f"sl{i % 2}",
                            name=f"ba{i % 2}", padded_shape=[128, 2 * npad]),
                    None)

        def allreduce_stats(ncols):
            aro = dr.tile([128, 4], dt.float32, addr_space="Shared",
                          name="aro", tag="aro")
            nc.sync.dma_start(arin[:, 0:4], stat[:, 0:4])
            nc.gpsimd.collective_compute(
                "AllReduce", OP.add, replica_groups=RG,
                ins=[arin[:, 0:4]], outs=[aro[:, 0:4]])
            nc.sync.dma_start(stat[:, 0:ncols], aro[:, 0:ncols])

        def stat_corr(nt, padv):
            nc.vector.tensor_scalar_mul(ptmp[:, 0:nt], padv, nlvS[:, 0:1])
            nc.vector.tensor_sub(stat[:, 0:nt], stat[:, 0:nt], ptmp[:, 0:nt])
            nc.vector.tensor_mul(ptmp[:, 0:nt], padv, padv)
            nc.vector.tensor_scalar_mul(ptmp[:, 0:nt], ptmp[:, 0:nt],
                                        nlvS[:, 0:1])
            nc.vector.tensor_sub(stat[:, nt:2 * nt], stat[:, nt:2 * nt],
                                 ptmp[:, 0:nt])

        def bn_ab(nt, gvec, bvec, bias):
            A = ab[:, 0:nt]; B = ab[:, 4:4 + nt]
            mu = ab[:, 2:2 + nt]; va = ab[:, 6:6 + nt]
            tmp = A
            nc.vector.tensor_scalar_mul(mu, stat[:, 0:nt], INV_N)
            nc.vector.tensor_scalar_mul(va, stat[:, nt:2 * nt], INV_N)
            if bias is not None:
                nc.vector.tensor_mul(tmp, mu, bias)
                nc.vector.tensor_scalar_mul(tmp, tmp, 2.0)
                nc.vector.tensor_add(va, va, tmp)
                nc.vector.tensor_mul(tmp, bias, bias)
                nc.vector.tensor_add(va, va, tmp)
                nc.vector.tensor_add(mu, mu, bias)
            nc.vector.tensor_mul(tmp, mu, mu)
            nc.vector.tensor_sub(va, va, tmp)
            nc.scalar.activation(va, va, AF.Sqrt, bias=eps_t[:, 0:1])
            nc.vector.reciprocal(va, va)
            nc.vector.tensor_mul(A, va, gvec)
            nc.vector.tensor_mul(mu, mu, A)
            nc.vector.tensor_sub(B, bvec, mu)

        # ---- atom encoder (h0 -> slot 1: layer 0's hT is fslot(1)) ----
        dlt = sb.tile([16, 128], dt.bfloat16, name="dlt")
        bas = sb.tile([128, 1], dt.float32, name="bas")
        nc.sync.dma_start(dlt[:], I['delta'].ap())
        nc.sync.dma_start(bas[:], I['base'].ap())
        hT = fslot(1)
        for ck in range(nchunk):
            xc = wk.tile([16, CH], dt.bfloat16, tag="xc")
            nc.sync.dma_start(xc[:], I['xt'].ap()[:, ts(ck, CH)])
            p0 = ps.tile([128, CH], dt.float32, space="PSUM", tag="mm")
            nc.tensor.matmul(p0[:], lhsT=dlt[0:9, :], rhs=xc[0:9, :],
                             start=True, stop=True)
            nc.scalar.activation(hT[:, ts(ck, CH)], p0[:], AF.Identity,
                                 bias=bas[:, 0:1], scale=1.0)
            nc.vector.tensor_reduce(sp[:, 0, ck:ck + 1], hT[:, ts(ck, CH)],
                                    axis=AX, op=OP.add)
            yd = wk.tile([128, CH], dt.float32, tag="yd")
            nc.scalar.activation(yd[:], hT[:, ts(ck, CH)], AF.Square,
                                 accum_out=sq[:, 0, ck:ck + 1])
        nc.vector.tensor_copy(hpad[:], bas[:])
        nc.vector.tensor_reduce(stat[:, 0:1], sp[:, 0, 0:nchunk], axis=AX,
                                op=OP.add)
        nc.vector.tensor_reduce(stat[:, 1:2], sq[:, 0, 0:nchunk], axis=AX,
                                op=OP.add)
        stat_corr(1, hpad[:])

        # ---- layers ----
        for l in range(n_layers):
            tl = tvals[l]
            nc.sync.dma_start(w1S[:], I['w1'].ap()[l])
            nc.sync.dma_start(w2S[:], I['w2'].ap()[l].rearrange(
                "(kt p) h -> p kt h", p=128))
            nc.sync.dma_start(w3S[:], I['w3'].ap()[l].rearrange(
                "(kt p) d -> p kt d", p=128))
            nc.sync.dma_start(v1S[:], I['v1'].ap()[l])
            nc.sync.dma_start(v2S[:], I['v2'].ap()[l])
            nc.sync.dma_start(v3S[:], I['v3'].ap()[l])
            for g in range(4):
                nc.sync.dma_start(c8S[32 * g:32 * g + 8, :],
                                  I['c8'].ap()[l * 8:(l + 1) * 8, :])

            if l == 0:
                nc.vector.tensor_copy(hhT[:], hT[:])
                nc.vector.tensor_copy(h2pad[:], hpad[:])
            else:
                allreduce_stats(2)
                bn_ab(1, v3P[:, 1:2], v3P[:, 2:3], None)
                nc.scalar.activation(hhT[:], hT[:], AF.Relu,
                                     bias=ab[:, 4:5], scale=ab[:, 0:1])
                nc.scalar.activation(h2pad[:], hpad[:], AF.Relu,
                                     bias=ab[:, 4:5], scale=ab[:, 0:1])
            nc.vector.tensor_copy(v3P[:], v3S[:])

            # spill h to DRAM; its slot becomes the fp32 denom accumulator
            nc.sync.dma_start(hdram[:], hT[:])

            for w in range(npad // 128):
                tw = wk.tile([128, 128], dt.float16, tag="tw")
                nc.sync.dma_start_transpose(tw[:], hhT[:, ts(w, 128)])
                nc.sync.dma_start(ag_in[ts(w, 128), :], tw[:])
            nc.sync.dma_start(ag_in[sent:sent + 128, :], sentS[:])
            ag_tab = dr.tile([NC * npad, 128], dt.float16, addr_space="Shared",
                             name="ag_tab", tag="agt")
            nc.gpsimd.collective_compute(
                "AllGather", OP.bypass, replica_groups=RG,
                ins=[ag_in[:]], outs=[ag_tab[:]])

            dnF = fslot(l + 1)        # aliases hT (after spill)
            nmF = fslot(l)            # the other slot
            nc.sync.dma_start(dnF[:], I['dneg'].ap())
            nc.vector.memset(nmF[:], 0.0)

            # edge phase
            for s in range(nsc):
                lo = s * SC
                scw = min(SC, slots_pad - lo)
                ntile = scw // 128
                gbuf = gp.tile([128, SC // 128, 128], dt.float16, tag="gbuf")
                for t in range(ntile):
                    nc.gpsimd.indirect_dma_start(
                        out=gbuf[:, t, :], out_offset=None, in_=ag_tab[:],
                        in_offset=bass.IndirectOffsetOnAxis(
                            ap=gix[:, (lo // 128) + t:(lo // 128) + t + 1],
                            axis=0))
                nc.sync.dma_start(
                    gdram[0:scw, :].rearrange("(t p) d -> p t d", p=128),
                    gbuf[:, 0:ntile, :])
                gT = gp.tile([128, SC], dt.float16, tag="gT")
                nc.sync.dma_start_transpose(gT[:, 0:scw], gdram[0:scw, :])
                ohsc = gp.tile([128, 2 * CH], dt.bfloat16, tag="ohsc")
                nc.sync.dma_start(ohsc[:], I['oh'].ap()[s])
                zF = gp.tile([128, SC], dt.float32, tag="zF")
                for cc in range(scw // CH):
                    pe = p1.tile([128, CH], dt.float32, space="PSUM", tag="pe")
                    nc.tensor.matmul(
                        pe[:], lhsT=c8S[32 * (cc % 4):32 * (cc % 4) + 8, :],
                        rhs=ohsc[32 * (cc % 4):32 * (cc % 4) + 8,
                                 ts(cc // 4, CH)],
                        start=True, stop=True,
                        tile_position=(32 * (cc % 4), 0))
                    nc.vector.tensor_add(zF[:, ts(cc, CH)], gT[:, ts(cc, CH)],
                                         pe[:])
                msgF = gp.tile([128, SC], dt.float32, tag="msgF")
                exF = gp.tile([128, SC], dt.float32, tag="exF")
                nc.vector.tensor_scalar_max(msgF[:, 0:scw], zF[:, 0:scw], 0.0)
                nc.scalar.activation(exF[:, 0:scw], zF[:, 0:scw], AF.Exp,
                                     scale=tl)
                nc.vector.tensor_scalar_max(exF[:, 0:scw], exF[:, 0:scw], 1.0)
                nc.vector.tensor_mul(msgF[:, 0:scw], msgF[:, 0:scw],
                                     exF[:, 0:scw])
                for (sl, nd, ln) in segs[s]:
                    nc.vector.tensor_add(dnF[:, nd:nd + ln],
                                         dnF[:, nd:nd + ln],
                                         exF[:, sl:sl + ln])
                    nc.vector.tensor_add(nmF[:, nd:nd + ln],
                                         nmF[:, nd:nd + ln],
                                         msgF[:, sl:sl + ln])

            # m (fp32) then hh (fp32, in nmF slot; dnF freed after)
            nc.vector.tensor_scalar_max(dnF[:], dnF[:], float(EPS))
            nc.vector.reciprocal(dnF[:], dnF[:])
            nc.vector.tensor_mul(nmF[:], nmF[:], dnF[:])
            for ck in range(nchunk):
                h2c = wk.tile([128, CH], dt.float32, tag="h2c")
                nc.sync.dma_start(h2c[:], hdram[:, ts(ck, CH)])
                if l > 0:
                    nc.scalar.activation(h2c[:], h2c[:], AF.Relu,
                                         bias=ab[:, 4:5], scale=ab[:, 0:1])
                nc.vector.tensor_add(nmF[:, ts(ck, CH)], nmF[:, ts(ck, CH)],
                                     h2c[:])
            hhF = nmF      # fp32 hh

            # ---- MLP ----
            for ht in range(2):
                for ck in range(nchunk):
                    pm = ps.tile([128, CH], dt.float32, space="PSUM", tag="mm")
                    nc.tensor.matmul(pm[:], lhsT=w1S[:, ts(ht, 128)],
                                     rhs=hhF[:, ts(ck, CH)],
                                     start=True, stop=True)
                    nc.vector.tensor_reduce(sp[:, ht, ck:ck + 1], pm[:],
                                            axis=AX, op=OP.add)
                    yd = wk.tile([128, CH], dt.float32, tag="yd")
                    nc.scalar.activation(yd[:], pm[:], AF.Square,
                                         accum_out=sq[:, ht, ck:ck + 1])
                    ysb = wk.tile([128, CH], dt.float32, tag="ysb")
                    nc.scalar.activation(ysb[:], pm[:], AF.Copy)
                    nc.sync.dma_start(y1d[ht, ck], ysb[:])
            nc.vector.tensor_reduce(stat[:, 0:2], sp[:, :, 0:nchunk], axis=AX,
                                    op=OP.add)
            nc.vector.tensor_reduce(stat[:, 2:4], sq[:, :, 0:nchunk], axis=AX,
                                    op=OP.add)
            nc.vector.tensor_copy(padb[:, 0:1], h2pad[:])
            pc = p1.tile([128, 2], dt.float32, space="PSUM", tag="pc")
            for ht in range(2):
                nc.tensor.matmul(pc[:, ht:ht + 1],
                                 lhsT=w1S[:, ts(ht, 128)].bitcast(dt.float32),
                                 rhs=h2pad[:], start=True, stop=True)
            nc.vector.tensor_copy(y1p[:], pc[:])
            stat_corr(2, y1p[:])
            allreduce_stats(4)
            bn_ab(2, v1S[:, 2:4], v1S[:, 4:6], v1S[:, 0:2])

            for kt in range(2):
                nc.scalar.activation(y1p[:, kt:kt + 1], y1p[:, kt:kt + 1],
                                     AF.Relu, bias=ab[:, 4 + kt:5 + kt],
                                     scale=ab[:, kt:kt + 1])
            pc2 = p1.tile([128, 2], dt.float32, space="PSUM", tag="pc")
            for hto in range(2):
                for kt in range(2):
                    nc.tensor.matmul(pc2[:, hto:hto + 1],
                                     lhsT=w2S[:, kt, ts(hto, 128)],
                                     rhs=y1p[:, kt:kt + 1],
                                     start=(kt == 0), stop=(kt == 1))
            nc.vector.tensor_copy(y2p[:], pc2[:])

            for ck in range(nchunk):
                a1 = []
                for kt in range(2):
                    ac = wk.tile([128, CH], dt.float32, tag=f"a1_{kt}")
                    nc.sync.dma_start(ac[:], y1d[kt, ck])
                    nc.scalar.activation(ac[:], ac[:], AF.Relu,
                                         bias=ab[:, 4 + kt:5 + kt],
                                         scale=ab[:, kt:kt + 1])
                    a1.append(ac)
                for hto in range(2):
                    pm2 = ps.tile([128, CH], dt.float32, space="PSUM", tag="mm")
                    for kt in range(2):
                        nc.tensor.matmul(pm2[:], lhsT=w2S[:, kt, ts(hto, 128)],
                                         rhs=a1[kt][:],
                                         start=(kt == 0), stop=(kt == 1))
                    nc.vector.tensor_reduce(sp[:, hto, ck:ck + 1], pm2[:],
                                            axis=AX, op=OP.add)
                    yd2 = wk.tile([128, CH], dt.float32, tag="yd")
                    nc.scalar.activation(yd2[:], pm2[:], AF.Square,
                                         accum_out=sq[:, hto, ck:ck + 1])
                    y2sb = wk.tile([128, CH], dt.float32, tag="ysb")
                    nc.scalar.activation(y2sb[:], pm2[:], AF.Copy)
                    nc.sync.dma_start(y2d[hto, ck], y2sb[:])
            nc.vector.tensor_reduce(stat[:, 0:2], sp[:, :, 0:nchunk], axis=AX,
                                    op=OP.add)
            nc.vector.tensor_reduce(stat[:, 2:4], sq[:, :, 0:nchunk], axis=AX,
                                    op=OP.add)
            stat_corr(2, y2p[:])
            allreduce_stats(4)
            bn_ab(2, v2S[:, 2:4], v2S[:, 4:6], v2S[:, 0:2])

            for kt in range(2):
                nc.scalar.activation(y2p[:, kt:kt + 1], y2p[:, kt:kt + 1],
                                     AF.Relu, bias=ab[:, 4 + kt:5 + kt],
                                     scale=ab[:, kt:kt + 1])
            pc3 = p1.tile([128, 2], dt.float32, space="PSUM", tag="pc")
            for kt in range(2):
                nc.tensor.matmul(pc3[:, 0:1], lhsT=w3S[:, kt, :],
                                 rhs=y2p[:, kt:kt + 1],
                                 start=(kt == 0), stop=(kt == 1))
            nc.vector.tensor_copy(o3p[:], pc3[:, 0:1])

            # out3 + residual: h_new lands in the hh slot (fp32)
            hN = hhF
            for ck in range(nchunk):
                a2 = []
                for kt in range(2):
                    a2c = wk.tile([128, CH], dt.float32, tag=f"a1_{kt}")
                    nc.sync.dma_start(a2c[:], y2d[kt, ck])
                    nc.scalar.activation(a2c[:], a2c[:], AF.Relu,
                                         bias=ab[:, 4 + kt:5 + kt],
                                         scale=ab[:, kt:kt + 1])
                    a2.append(a2c)
                pm3 = ps.tile([128, CH], dt.float32, space="PSUM", tag="mm")
                nc.tensor.matmul(pm3[:], lhsT=w3S[:, 0, :],
                                 rhs=a2[0][:], start=True,
                                 stop=False)
                nc.tensor.matmul(pm3[:], lhsT=w3S[:, 1, :],
                                 rhs=a2[1][:], start=False,
                                 stop=True)
                if l == 0:
                    nc.vector.tensor_copy(hN[:, ts(ck, CH)], pm3[:])
                else:
                    hc = wk.tile([128, CH], dt.float32, tag="h2c")
                    nc.sync.dma_start(hc[:], hdram[:, ts(ck, CH)])
                    nc.vector.tensor_add(hc[:], hc[:], pm3[:])
                    nc.vector.tensor_copy(hN[:, ts(ck, CH)], hc[:])
            nc.vector.tensor_scalar_add(hN[:], hN[:], v3S[:, 0:1])
            if l == 0:
                nc.vector.tensor_copy(hpad[:], o3p[:])
            else:
                nc.vector.tensor_add(hpad[:], hpad[:], o3p[:])
            nc.vector.tensor_add(hpad[:], hpad[:], v3S[:, 0:1])
            for ck in range(nchunk):
                nc.vector.tensor_reduce(sp[:, 0, ck:ck + 1], hN[:, ts(ck, CH)],
                                        axis=AX, op=OP.add)
                yd3 = wk.tile([128, CH], dt.float32, tag="yd")
                nc.scalar.activation(yd3[:], hN[:, ts(ck, CH)], AF.Square,
                                     accum_out=sq[:, 0, ck:ck + 1])
            nc.vector.tensor_reduce(stat[:, 0:1], sp[:, 0, 0:nchunk], axis=AX,
                                    op=OP.add)
            nc.vector.tensor_reduce(stat[:, 1:2], sq[:, 0, 0:nchunk], axis=AX,
                                    op=OP.add)
            stat_corr(1, hpad[:])
            hT = hN

        # ---- final bn + pooling ----
        allreduce_stats(2)
        bn_ab(1, v3P[:, 1:2], v3P[:, 2:3], None)
        nc.scalar.activation(hhT[:], hT[:], AF.Identity, bias=ab[:, 4:5],
                             scale=ab[:, 0:1])
        for w in range(npad // 128):
            tw = wk.tile([128, 128], dt.float16, tag="tw")
            nc.sync.dma_start_transpose(tw[:], hhT[:, ts(w, 128)])
            nc.sync.dma_start(ag_in[ts(w, 128), :], tw[:])
        zeroS = wk.tile([128, 128], dt.float16, tag="tw")
        nc.vector.memset(zeroS[:], 0.0)
        nc.sync.dma_start(ag_in[sent:sent + 128, :], zeroS[:])
        nc.vector.memset(poolT[:], 0.0)
        for s in range(meta['pnsc']):
            lo = s * SC
            scw = min(SC, pslots_pad - lo)
            ntile = scw // 128
            gbuf = gp.tile([128, SC // 128, 128], dt.float16, tag="gbuf")
            for t in range(ntile):
                nc.gpsimd.indirect_dma_start(
                    out=gbuf[:, t, :], out_offset=None, in_=ag_in[:],
                    in_offset=bass.IndirectOffsetOnAxis(
                        ap=pix[:, (lo // 128) + t:(lo // 128) + t + 1],
                        axis=0))
            nc.sync.dma_start(
                gdram[0:scw, :].rearrange("(t p) d -> p t d", p=128),
                gbuf[:, 0:ntile, :])
            gT = gp.tile([128, SC], dt.float16, tag="gT")
            nc.sync.dma_start_transpose(gT[:, 0:scw], gdram[0:scw, :])
            for (sl, nd, ln) in psegs[s]:
                nc.vector.tensor_add(poolT[:, nd:nd + ln],
                                     poolT[:, nd:nd + ln], gT[:, sl:sl + ln])
        nc.sync.dma_start(out_pool.ap(), poolT[:])
        ctx.close()
    nc.compile()
    return nc


# ---------------------------------------------------------------------------
# run
# ---------------------------------------------------------------------------

def _run_device(inputs):
    import ml_dtypes
    from concourse import bass_utils
    n_layers = int(os.environ.get("GCN_LAYERS", N_LAYERS))
    meta, shared, cores = _preprocess(inputs, n_layers)

    key = ("prog", os.environ.get("GCN_STAGE", "9"), n_layers, meta['slots_pad'], meta['pslots_pad'],
           str(meta['segs']), str(meta['psegs']), meta['npad'])
    if key not in _CACHE:
        t0 = time.time()
        _CACHE[key] = _build_program(meta)
        print(f"[gcn] build+compile: {time.time()-t0:.1f}s", file=sys.stderr)
    nc = _CACHE[key]

    bf = ml_dtypes.bfloat16
    in_maps = []
    for c in range(NC):
        cd = cores[c]
        in_maps.append(dict(
            xt=cd['xt'].astype(bf), gidx=cd['gidx'],
            oh=cd['oh'].astype(bf), pool_idx=cd['pool_idx'],
            dneg=cd['dneg'].astype(np.float32), nlv=cd['nlv'],
            delta=shared['delta'].astype(bf), base=shared['base'].astype(np.float32),
            c8=shared['c8'].astype(bf),
            w1=shared['w1'].astype(np.float32), w2=shared['w2'].astype(np.float32),
            w3=shared['w3'].astype(np.float32),
            v1=shared['v1'].astype(np.float32), v2=shared['v2'].astype(np.float32),
            v3=shared['v3'].astype(np.float32),
        ))
    t0 = time.time()
    res = bass_utils.run_bass_kernel_spmd(nc, in_maps, core_ids=list(range(NC)))
    ns = int((time.time() - t0) * 1e9)

    out = np.zeros((G, D), np.float32)
    for c in range(NC):
        pooled = res.results[c]['out_pool'].astype(np.float32)   # [128, GPC]
        cd = cores[c]
        cnt = np.maximum(cd['gsz'][cd['gorder']], 1).astype(np.float32)
        hg = (pooled / cnt[None, :]).T                            # [GPC, 128]
        out[c * GPC + cd['gorder']] = hg
    res_out = out @ inputs['predW'].astype(np.float32) + inputs['predb'].astype(np.float32)
    return res_out.astype(np.float32), ns


def kernel(**inputs):
    inputs = {k: np.asarray(v) for k, v in inputs.items()}
    if os.environ.get("GCN_FORCE_HOST"):
        return _host_reference(**inputs)
    try:
        global LAST_HW_EXEC_NS
        out, ns = _run_device(inputs)
        LAST_HW_EXEC_NS = ns
        return out
    except Exception:
        traceback.print_exc()
        print("device path failed; falling back to host", file=sys.stderr)
        return _host_reference(**inputs)
